# revision 6
# baseline (speedup 1.0000x reference)
"""Multi-head attention kernel for 8 Trainium2 NeuronCores.

Problem: B=4, N=2048, C=1024, H=16 heads, d=64, fp32 in/out.
Sharding: core c -> batch c//2, heads (c%2)*8 .. +8  (8 (b,h) pairs per core).
Each core computes full attention for its head slice independently.

Per-core pipeline (per head):
  - gpsimd cast-DMA loads Q (duplicated twice along free dim), K, V as bf16.
  - xbar DMA-transposes build Q^T duplicated on both partition halves and
    K^T with even key-blocks on partitions 0-63, odd on 64-127.
  - QK^T: row-packed matmul pairs (tile_position (0,0)/(64,0)) compute two
    key-blocks concurrently (contraction d=64 only fills half the PE array).
  - exp on ScalarE, scale=1/8 fused into the activation, bf16 out.
  - PV: V augmented with a ones column (65 cols) so the PV matmul also
    produces the softmax denominators; accumulated over key blocks in PSUM.
  - PE-transpose of ctx^T [65,128] blocks -> [128,65], then reciprocal +
    tensor_scalar multiply normalizes and stores into the staging tile.
  - Staged [128, 512] tiles DMA'd to DRAM at the end.
"""

import numpy as np

import concourse.bass as bass
from concourse import bacc
import concourse.mybir as mybir
import concourse.tile as tile
from concourse.masks import make_identity

F32 = mybir.dt.float32
BF16 = mybir.dt.bfloat16

# Full-problem constants (hardcoded; kernel.py must be self-contained).
B = 4
N = 2048
C = 1024
H_TOTAL = 16
D = 64
N_CORES = 8
H_LOC = 8          # heads per core
C_LOC = H_LOC * D  # 512: dram cols per core
SCALE = 0.125      # 1/sqrt(64)


def build_nc(h_loc=H_LOC, n_q=N, n_k=N):
    """Build the single-core Bass program (SPMD: same NEFF on all 8 cores)."""
    nc = bacc.Bacc("TRN2", target_bir_lowering=False)

    qb_n = n_q // 128          # query blocks
    kb_n = n_k // 128          # key blocks
    kbp_n = kb_n // 2          # key block pairs
    qq_n = n_q // 512          # query chunks of 512
    c_loc = h_loc * D

    q_d = nc.dram_tensor("query_layer", [n_q, c_loc], F32, kind="ExternalInput")
    k_d = nc.dram_tensor("key_layer", [n_k, c_loc], F32, kind="ExternalInput")
    v_d = nc.dram_tensor("value_layer", [n_k, c_loc], F32, kind="ExternalInput")
    o_d = nc.dram_tensor("out", [n_q, c_loc], F32, kind="ExternalOutput")

    with tile.TileContext(nc) as tc:
        with (
            tc.tile_pool(name="persist", bufs=1) as persist,
            tc.tile_pool(name="qscratch", bufs=2) as qscratch,
            tc.tile_pool(name="ppool", bufs=3) as ppool,
            tc.tile_pool(name="ctxsb", bufs=2) as ctxsb,
            tc.tile_pool(name="rpool", bufs=2) as rpool,
            tc.tile_pool(name="spool", bufs=2, space="PSUM") as spool,
            tc.tile_pool(name="ctxps", bufs=2, space="PSUM") as ctxps,
            tc.tile_pool(name="trps", bufs=2, space="PSUM") as trps,
        ):
            ident = persist.tile([128, 128], F32, name="ident")
            make_identity(nc, ident)

            # ---- per-head input prep (cast loads + xbar transposes) ----
            qn = {}
            kn = {}
            va = {}
            q2t = {}
            k2t = {}
            for h in range(h_loc):
                # natural-layout bf16 tiles
                qn[h] = persist.tile([128, qb_n, 2, D], BF16, name=f"qn{h}")
                kn[h] = persist.tile([128, kb_n, D], BF16, name=f"kn{h}")
                va[h] = persist.tile([128, kb_n, D + 1], BF16, name=f"va{h}")

                q_src = q_d[:, h * D:(h + 1) * D].rearrange(
                    "(blk p) d -> p blk d", p=128)
                k_src = k_d[:, h * D:(h + 1) * D].rearrange(
                    "(blk p) d -> p blk d", p=128)
                v_src = v_d[:, h * D:(h + 1) * D].rearrange(
                    "(blk p) d -> p blk d", p=128)

                # f32 -> bf16 casting DMAs (SWDGE). Q is duplicated (the two
                # copies feed the two row-groups of the packed QK matmuls)
                # via a single DVE broadcast-copy so each tile has exactly
                # one writer -- the xbar transpose only gets one sync wait.
                q1 = qscratch.tile([128, qb_n, D], BF16, name="q1")
                nc.gpsimd.dma_start(out=q1, in_=q_src)
                q1_dup = bass.AP(
                    tensor=q1.tensor,
                    offset=q1.offset,
                    ap=[q1.ap[0], q1.ap[1], [0, 2], q1.ap[2]],
                )
                nc.vector.tensor_copy(qn[h], q1_dup)
                nc.gpsimd.dma_start(out=kn[h], in_=k_src)
                nc.gpsimd.dma_start(out=va[h][:, :, 0:D], in_=v_src)
                nc.vector.memset(va[h][:, :, D], 1.0)

                # transposed layouts via xbar DMA transpose (bf16 SBUF->SBUF)
                q2t[h] = persist.tile([128, qb_n, 128], BF16, name=f"q2t{h}")
                k2t[h] = persist.tile([128, kbp_n, 128], BF16, name=f"k2t{h}")
                for qb in range(qb_n):
                    # in [128 seq, (2 dup x 64 d)] -> out [128 (dup d), 128 seq]
                    nc.sync.dma_start(
                        out=q2t[h][:, qb, :], in_=qn[h][:, qb, :, :],
                        transpose=True)
                for kbp in range(kbp_n):
                    # in [128 seq, (2 kb x 64 d)] -> out rows 0-63 = K^T[2kbp],
                    # rows 64-127 = K^T[2kbp+1]
                    nc.sync.dma_start(
                        out=k2t[h][:, kbp, :], in_=kn[h][:, kbp * 2:kbp * 2 + 2, :],
                        transpose=True)

            # ---- output staging tiles ----
            outst = [
                persist.tile([128, c_loc], F32, name=f"outst{qb}")
                for qb in range(qb_n)
            ]

            # ---- main loop ----
            for h in range(h_loc):
                for qq in range(qq_n):
                    ctx = ctxps.tile([D + 1, 512], F32, name="ctx")
                    for kbp in range(kbp_n):
                        s = spool.tile([128, 1024], F32, name="sgran")
                        # packed QK^T: even kb on rows 0-63, odd kb on 64-127
                        nc.tensor.matmul(
                            s[:, 0:512],
                            lhsT=k2t[h][0:64, kbp, :],
                            rhs=q2t[h][0:64, qq * 4:qq * 4 + 4, :],
                            start=True, stop=True,
                            tile_position=(0, 0))
                        nc.tensor.matmul(
                            s[:, 512:1024],
                            lhsT=k2t[h][64:128, kbp, :],
                            rhs=q2t[h][64:128, qq * 4:qq * 4 + 4, :],
                            start=True, stop=True,
                            tile_position=(64, 0))
                        p = ppool.tile([128, 1024], BF16, name="p")
                        nc.scalar.activation(p, s,
                                             mybir.ActivationFunctionType.Exp,
                                             scale=SCALE)
                        # PV: accumulate ctx^T_aug [65, 512] over key blocks
                        nc.tensor.matmul(
                            ctx,
                            lhsT=va[h][:, kbp * 2, :],
                            rhs=p[:, 0:512],
                            start=(kbp == 0), stop=False)
                        nc.tensor.matmul(
                            ctx,
                            lhsT=va[h][:, kbp * 2 + 1, :],
                            rhs=p[:, 512:1024],
                            start=False, stop=(kbp == kbp_n - 1))

                    # drain ctx: PSUM -> SBUF, transpose, normalize, stage
                    ctxt = ctxsb.tile([D + 1, 512], F32, name="ctxt")
                    nc.vector.tensor_copy(ctxt, ctx)
                    tr = trps.tile([128, 4, D + 1], F32, name="tr")
                    for t in range(4):
                        nc.tensor.transpose(
                            tr[:, t, :], ctxt[:, t * 128:(t + 1) * 128],
                            ident[0:D + 1, 0:D + 1])
                    rs = rpool.tile([128, 4], F32, name="rs")
                    for t in range(4):
                        nc.vector.reciprocal(rs[:, t:t + 1], tr[:, t, D:D + 1])
                        qb = qq * 4 + t
                        nc.vector.tensor_scalar_mul(
                            outst[qb][:, h * D:(h + 1) * D],
                            tr[:, t, 0:D],
                            rs[:, t:t + 1])

            # ---- output DMA ----
            for qb in range(qb_n):
                nc.sync.dma_start(
                    out=o_d[qb * 128:(qb + 1) * 128, :], in_=outst[qb])

    nc.finalize()
    return nc


_NC_CACHE = {}


def _get_nc():
    if "nc" not in _NC_CACHE:
        _NC_CACHE["nc"] = build_nc()
    return _NC_CACHE["nc"]


def _shard(x, c):
    b = c // 2
    cs = (c % 2) * C_LOC
    return np.ascontiguousarray(x[b, :, cs:cs + C_LOC], dtype=np.float32)


def run_spmd(query_layer, key_layer, value_layer, **kwargs):
    """Run on 8 cores; returns (full_output, BassKernelResults)."""
    from concourse.bass_utils import run_bass_kernel_spmd

    q = np.asarray(query_layer, dtype=np.float32)
    k = np.asarray(key_layer, dtype=np.float32)
    v = np.asarray(value_layer, dtype=np.float32)
    in_maps = [
        {"query_layer": _shard(q, c), "key_layer": _shard(k, c),
         "value_layer": _shard(v, c)}
        for c in range(N_CORES)
    ]
    nc = _get_nc()
    res = run_bass_kernel_spmd(nc, in_maps, core_ids=list(range(N_CORES)),
                               **kwargs)
    out = np.empty((B, N, C), dtype=np.float32)
    for c in range(N_CORES):
        b = c // 2
        cs = (c % 2) * C_LOC
        out[b, :, cs:cs + C_LOC] = res.results[c]["out"]
    return out, res


def kernel(query_layer, key_layer, value_layer):
    out, _ = run_spmd(query_layer, key_layer, value_layer)
    return out


# revision 8
# speedup vs baseline: 1.4499x; 1.4499x over previous
"""Multi-head attention kernel for 8 Trainium2 NeuronCores.

Problem: B=4, N=2048, C=1024, H=16 heads, d=64, fp32 in/out.
Sharding: core c -> batch c//2, heads (c%2)*8 .. +8  (8 (b,h) pairs per core).
Each core computes full attention for its head slice independently.

Per-core pipeline (per head, all matmuls bf16 with fp32 PSUM):
  - gpsimd cast-DMA loads Q/K/V as bf16 (Q duplicated via a DVE
    broadcast-copy so both PE row-groups can stream it).
  - one batched xbar DMA-transpose per tensor builds Q^T (duplicated on
    both partition halves) and K^T (even key-blocks on partitions 0-63,
    odd on 64-127).
  - QK^T: row-packed matmul pairs (tile_position (0,0)/(64,0)) compute two
    key-blocks concurrently (contraction d=64 fills half the PE array each).
  - exp on ScalarE over 3-bank granules (F=1536), scale=1/8 fused, bf16 out.
  - PV: V augmented with a ones column (65 cols) so the PV matmul also
    produces the softmax denominators; accumulated over key blocks in PSUM.
  - ctx drain: cast-copy to bf16, batched xbar transpose, reciprocal of the
    sums column, fused normalize-multiply into the staging tile.
  - Staged [128, 512] rows DMA'd to DRAM per query-block.
"""

import numpy as np

import concourse.bass as bass
from concourse import bacc
import concourse.mybir as mybir
import concourse.tile as tile

F32 = mybir.dt.float32
BF16 = mybir.dt.bfloat16

# Full-problem constants (hardcoded; kernel.py must be self-contained).
B = 4
N = 2048
C = 1024
H_TOTAL = 16
D = 64
N_CORES = 8
H_LOC = 8          # heads per core
C_LOC = H_LOC * D  # 512: dram cols per core
SCALE = 0.125      # 1/sqrt(64)
GRAN = 3           # S granule size in 512-col units (3 PSUM banks)


def build_nc(h_loc=H_LOC, n_q=N, n_k=N):
    """Build the single-core Bass program (SPMD: same NEFF on all 8 cores)."""
    nc = bacc.Bacc("TRN2", target_bir_lowering=False)

    qb_n = n_q // 128          # query blocks
    kb_n = n_k // 128          # key blocks
    kbp_n = kb_n // 2          # key block pairs
    qq_n = n_q // 512          # query chunks of 512
    c_loc = h_loc * D

    q_d = nc.dram_tensor("query_layer", [n_q, c_loc], F32, kind="ExternalInput")
    k_d = nc.dram_tensor("key_layer", [n_k, c_loc], F32, kind="ExternalInput")
    v_d = nc.dram_tensor("value_layer", [n_k, c_loc], F32, kind="ExternalInput")
    o_d = nc.dram_tensor("out", [n_q, c_loc], F32, kind="ExternalOutput")

    with tile.TileContext(nc) as tc:
        with (
            tc.tile_pool(name="persist", bufs=1) as persist,
            tc.tile_pool(name="qscratch", bufs=2) as qscratch,
            tc.tile_pool(name="ppool", bufs=3) as ppool,
            tc.tile_pool(name="ctxsb", bufs=2) as ctxsb,
            tc.tile_pool(name="trsbp", bufs=2) as trsbp,
            tc.tile_pool(name="rpool", bufs=2) as rpool,
            tc.tile_pool(name="spool", bufs=2, space="PSUM") as spool,
            tc.tile_pool(name="ctxps", bufs=2, space="PSUM") as ctxps,
        ):
            qn = {}
            kn = {}
            va = {}
            q2t = {}
            k2t = {}

            def prep(h):
                """Cast-load + transpose inputs for head h."""
                qn[h] = persist.tile([128, qb_n, 2, D], BF16, name=f"qn{h}")
                kn[h] = persist.tile([128, kb_n, D], BF16, name=f"kn{h}")
                va[h] = persist.tile([128, kb_n, D + 1], BF16, name=f"va{h}")

                q_src = q_d[:, h * D:(h + 1) * D].rearrange(
                    "(blk p) d -> p blk d", p=128)
                k_src = k_d[:, h * D:(h + 1) * D].rearrange(
                    "(blk p) d -> p blk d", p=128)
                v_src = v_d[:, h * D:(h + 1) * D].rearrange(
                    "(blk p) d -> p blk d", p=128)

                # f32 -> bf16 casting DMAs (SWDGE). Q duplicated via a DVE
                # broadcast copy (single writer per tile keeps xbar waits low).
                q1 = qscratch.tile([128, qb_n, D], BF16, name="q1")
                nc.gpsimd.dma_start(out=q1, in_=q_src)
                q1_dup = bass.AP(
                    tensor=q1.tensor,
                    offset=q1.offset,
                    ap=[q1.ap[0], q1.ap[1], [0, 2], q1.ap[2]],
                )
                nc.vector.tensor_copy(qn[h], q1_dup)
                nc.gpsimd.dma_start(out=kn[h], in_=k_src)
                nc.gpsimd.dma_start(out=va[h][:, :, 0:D], in_=v_src)
                nc.vector.memset(va[h][:, :, D], 1.0)

                # batched xbar transposes: out[:, blk, :] = in[:, 128b:+128].T
                q2t[h] = persist.tile([128, qb_n, 128], BF16, name=f"q2t{h}")
                k2t[h] = persist.tile([128, kbp_n, 128], BF16, name=f"k2t{h}")
                nc.sync.dma_start_transpose(q2t[h], qn[h])
                nc.sync.dma_start_transpose(k2t[h], kn[h])

            # output staging: [128, qb, c] so one fused normalize-mul can
            # write 4 query blocks at once
            outst = persist.tile([128, qb_n, c_loc], F32, name="outst")

            prep(0)
            if h_loc > 1:
                prep(1)

            # ---- main loop: global stream of 512-col (h, qq, kb) units ----
            units = [(h, qq, kb)
                     for h in range(h_loc)
                     for qq in range(qq_n)
                     for kb in range(kb_n)]

            def drain(h, qq):
                """Normalize ctx^T_aug and stage the result."""
                ctx = ctx_tiles.pop((h, qq))
                ctxt = ctxsb.tile([80, 512], BF16, name="ctxt")
                nc.vector.memset(ctxt[64:80, :], 0.0)
                nc.vector.tensor_copy(ctxt[0:65, :], ctx)
                trsb = trsbp.tile([128, 4, 80], BF16, name="trsb")
                nc.sync.dma_start_transpose(trsb, ctxt)
                rs = rpool.tile([128, 4], F32, name="rs")
                nc.vector.reciprocal(rs, trsb[:, :, D])
                rs_b = bass.AP(
                    tensor=rs.tensor,
                    offset=rs.offset,
                    ap=[rs.ap[0], rs.ap[1], [0, D]],
                )
                nc.vector.scalar_tensor_tensor(
                    out=outst[:, qq * 4:qq * 4 + 4, h * D:(h + 1) * D],
                    in0=trsb[:, :, 0:D],
                    scalar=1.0,
                    in1=rs_b,
                    op0=mybir.AluOpType.bypass,
                    op1=mybir.AluOpType.mult,
                )

            ctx_tiles = {}
            n_units = len(units)
            u = 0
            while u < n_units:
                group = units[u:u + GRAN]
                g = len(group)
                gr = spool.tile([128, GRAN * 512], F32, name="sgran")
                psb = ppool.tile([128, GRAN * 512], BF16, name="p")
                # QK matmuls for the group (kb pairs stay emission-adjacent)
                for j, (h, qq, kb) in enumerate(group):
                    half = kb % 2
                    nc.tensor.matmul(
                        gr[:, j * 512:(j + 1) * 512],
                        lhsT=k2t[h][half * 64:half * 64 + 64, kb // 2, :],
                        rhs=q2t[h][half * 64:half * 64 + 64,
                                   qq * 4:qq * 4 + 4, :],
                        start=True, stop=True,
                        tile_position=(half * 64, 0))
                # exp over the whole granule
                nc.scalar.activation(psb[:, 0:g * 512], gr[:, 0:g * 512],
                                     mybir.ActivationFunctionType.Exp,
                                     scale=SCALE)
                # PV accumulation per unit
                for j, (h, qq, kb) in enumerate(group):
                    if kb == 0:
                        ctx_tiles[(h, qq)] = ctxps.tile(
                            [D + 1, 512], F32, name="ctx")
                    nc.tensor.matmul(
                        ctx_tiles[(h, qq)],
                        lhsT=va[h][:, kb, :],
                        rhs=psb[:, j * 512:(j + 1) * 512],
                        start=(kb == 0), stop=(kb == kb_n - 1))
                    if kb == kb_n - 1:
                        drain(h, qq)
                    # emit prep for head h+2 mid-way through head h
                    if qq == 1 and kb == 15 and h + 2 < h_loc:
                        prep(h + 2)
                u += g

            # ---- output DMA ----
            for qb in range(qb_n):
                nc.sync.dma_start(
                    out=o_d[qb * 128:(qb + 1) * 128, :], in_=outst[:, qb, :])

    nc.finalize()
    return nc


_NC_CACHE = {}


def _get_nc():
    if "nc" not in _NC_CACHE:
        _NC_CACHE["nc"] = build_nc()
    return _NC_CACHE["nc"]


def _shard(x, c):
    b = c // 2
    cs = (c % 2) * C_LOC
    return np.ascontiguousarray(x[b, :, cs:cs + C_LOC], dtype=np.float32)


def run_spmd(query_layer, key_layer, value_layer, **kwargs):
    """Run on 8 cores; returns (full_output, BassKernelResults)."""
    from concourse.bass_utils import run_bass_kernel_spmd

    q = np.asarray(query_layer, dtype=np.float32)
    k = np.asarray(key_layer, dtype=np.float32)
    v = np.asarray(value_layer, dtype=np.float32)
    in_maps = [
        {"query_layer": _shard(q, c), "key_layer": _shard(k, c),
         "value_layer": _shard(v, c)}
        for c in range(N_CORES)
    ]
    nc = _get_nc()
    res = run_bass_kernel_spmd(nc, in_maps, core_ids=list(range(N_CORES)),
                               **kwargs)
    out = np.empty((B, N, C), dtype=np.float32)
    for c in range(N_CORES):
        b = c // 2
        cs = (c % 2) * C_LOC
        out[b, :, cs:cs + C_LOC] = res.results[c]["out"]
    return out, res


def kernel(query_layer, key_layer, value_layer):
    out, _ = run_spmd(query_layer, key_layer, value_layer)
    return out
